# revision 15
# baseline (speedup 1.0000x reference)
"""CharCNN encoder kernel for Trainium2 (8 NeuronCores, data-parallel).

Strategy (per core, 4096 tokens = 98304 chars):
  - ids arrive as a single [1, 98308] uint8 row (96 KB/core, not
    partition-replicated on host); ACT converts to bf16 and a K=1
    ones-matmul broadcasts each 388-char chunk across the 128
    partitions into PSUM.
  - one-hot gather: OH[v,c] = (ids[c]==v) built on DVE (is_equal of the
    f32 PSUM broadcast vs an f32 iota), then E = emb_table.T @ OH on the
    PE (gather-as-matmul, K=128 vocab).
  - two shifted gather matmuls build a 2-band im2col directly in PSUM:
    rows [0:30) = E[:,c], rows [32:62) = E[:,c+1] (offset 32 required by
    PE tile_position rules; gap rows zeroed via zero-padded stationary).
  - conv = 3 bf16 matmuls on the im2col (K<=68) with mask rows (-1e9 at
    invalid window positions) and a ones row (bias) folded into the
    stationary operand.
  - max-pool = DVE windowed reduce_max (window 24, poisoned tails lose).
  - PE transpose + ACT relu-copies assemble (token, 150) rows quantized
    to uint8 (y*32, rounds to nearest on the store); the host divides
    by 32. Quantization adds ~0.5% relative error against a 2% budget
    and quarters the device->host fetch vs f32.

Host side: one jax.jit(shard_map(bass_exec)) callable is built and
cached per weight-set; warm calls reuse the compiled executable, so the
per-call cost is ids H2D (0.8 MB total) + execute + uint8 output D2H
(4.9 MB) — all dominated by the transport round trips, not device time.
"""

import numpy as np
import ml_dtypes

BF16 = ml_dtypes.bfloat16

VOCAB = 128
D = 30  # embed
F = 50  # filters per ksize
B, S, C = 64, 512, 24
N_CORES = 8
TOK_PER_CORE = (B // N_CORES) * S  # 4096
CHARS_PER_CORE = TOK_PER_CORE * C  # 98304

CHUNK_TOK = 16          # tokens per chunk
CHUNK = CHUNK_TOK * C   # 384 chars per chunk
SB_CHUNKS = 4           # chunks per superblock
SB_TOK = SB_CHUNKS * CHUNK_TOK  # 64 tokens
N_SB = TOK_PER_CORE // SB_TOK   # 64 superblocks
IDS_STRIDE = SB_CHUNKS * CHUNK  # 1536
IDS_W = IDS_STRIDE + 4          # 1540 (4-char halo for shifted reads)
IDS_LEN = CHARS_PER_CORE + 4    # 98308

NEG = -1.0e9
QSCALE = 32.0  # uint8 output quantization: u8 = Relu(32*x), max |y| ~3.2 << 8

_CACHE = {}


def _host_constants(emb_table, w2, b2, w3, b3, w4, b4):
    """Pack conv weights into PE stationary operands (see kernel docstring)."""
    emb = np.asarray(emb_table, np.float32)
    w2 = np.asarray(w2, np.float32)
    w3 = np.asarray(w3, np.float32)
    w4 = np.asarray(w4, np.float32)
    b2 = np.asarray(b2, np.float32)
    b3 = np.asarray(b3, np.float32)
    b4 = np.asarray(b4, np.float32)

    # gather stationary: (vocab, 32), cols 30:32 zero
    tableT = np.zeros((VOCAB, 32), np.float32)
    tableT[:, :D] = emb

    # im2col row layout (68 rows):
    #   0:30   band0 = E[:, c]      (j=0)
    #   30:32  zero
    #   32:62  band1 = E[:, c+1]    (j=1)
    #   62:64  zero
    #   64     mask l==21, 65 mask l==22, 66 mask l==23, 67 ones (bias)
    # T1 col layout: 0:50 y3 | 50:100 y4 | 100:128 y2a (w2 filters 0:28)
    sA = np.zeros((68, 128), np.float32)
    for j in (0, 1):
        r = 32 * j
        # w?[f, d, j] -> rows r+d, col f
        sA[r : r + D, 0:50] = w3[:, :, j].T
        sA[r : r + D, 50:100] = w4[:, :, j].T
        sA[r : r + D, 100:128] = w2[:28, :, j].T
    sA[64, 50:100] = NEG            # l=21 invalid for k=4
    sA[65, 0:100] = NEG             # l=22 invalid for k=3,4
    sA[66, 0:128] = NEG             # l=23 invalid for all
    sA[67, 0:50] = b3
    sA[67, 50:100] = b4
    sA[67, 100:128] = b2[:28]

    # y2b = w2 filters 28:50, padded to 32 cols
    sB = np.zeros((68, 32), np.float32)
    for j in (0, 1):
        r = 32 * j
        sB[r : r + D, 0:22] = w2[28:, :, j].T
    sB[66, 0:22] = NEG
    sB[67, 0:22] = b2[28:]

    # shift-2 stationary: rhs = ims[0:62, c+2] -> rows 0:30 = E[:,c+2],
    # rows 32:62 = E[:,c+3]. cols 0:50 y3 (j=2), 50:100 y4 (j=2,3).
    sC = np.zeros((62, 100), np.float32)
    sC[0:D, 0:50] = w3[:, :, 2].T
    sC[0:D, 50:100] = w4[:, :, 2].T
    sC[32 : 32 + D, 50:100] = w4[:, :, 3].T

    # mask/ones rows DMA'd once into the persistent im2col tiles
    cc = np.arange(CHUNK + 2, dtype=np.int64) % C
    masks = np.zeros((4, CHUNK + 2), np.float32)
    masks[0] = (cc == 21).astype(np.float32)
    masks[1] = (cc == 22).astype(np.float32)
    masks[2] = (cc == 23).astype(np.float32)
    masks[3] = 1.0

    iota2d = np.broadcast_to(
        np.arange(VOCAB, dtype=np.float32).reshape(VOCAB, 1), (VOCAB, CHUNK + 4)
    )
    ident = np.eye(128, dtype=np.float32)
    ones_row = np.ones((1, 128), np.float32)

    return {
        "tableT": tableT.astype(BF16),
        "sA": sA.astype(BF16),
        "sB": sB.astype(BF16),
        "sC": sC.astype(BF16),
        "masks": masks.astype(BF16),
        "iota2d": np.ascontiguousarray(iota2d, dtype=np.float32),
        "ident": ident,
        "ones_row": ones_row.astype(BF16),
    }


def _build(consts, n_sb=N_SB):
    import concourse.mybir as mybir
    from concourse import bacc
    from concourse.tile import TileContext

    f32 = mybir.dt.float32
    u8 = mybir.dt.uint8
    bf16 = mybir.dt.bfloat16
    W = CHUNK  # 384

    nc = bacc.Bacc(name="charcnn")
    ids_d = nc.dram_tensor("ids", [1, IDS_LEN], u8, kind="ExternalInput")
    out_d = nc.dram_tensor("out", [n_sb * SB_TOK, 150], u8, kind="ExternalOutput")

    tableT_d = nc.inline_tensor(consts["tableT"], "tableT")
    sA_d = nc.inline_tensor(consts["sA"], "sA")
    sB_d = nc.inline_tensor(consts["sB"], "sB")
    sC_d = nc.inline_tensor(consts["sC"], "sC")
    masks_d = nc.inline_tensor(consts["masks"], "masks")
    iota_d = nc.inline_tensor(consts["iota2d"], "iota2d")
    ident_d = nc.inline_tensor(consts["ident"], "ident")
    ones_d = nc.inline_tensor(consts["ones_row"], "ones_row")

    with TileContext(nc) as tc:
        with (
            tc.tile_pool(name="consts", bufs=1) as cpool,
            tc.tile_pool(name="idsp", bufs=2) as idpool,
            tc.tile_pool(name="ohp", bufs=3) as ohpool,
            tc.tile_pool(name="imsp", bufs=1) as imspool,
            tc.tile_pool(name="stage", bufs=2) as stpool,
            tc.tile_pool(name="outp", bufs=2) as outpool,
            tc.tile_pool(name="pbc", bufs=2, space="PSUM") as pbc,
            tc.tile_pool(name="pim", bufs=1, space="PSUM") as pim,
            tc.tile_pool(name="pt1", bufs=2, space="PSUM") as pt1,
            tc.tile_pool(name="pt2", bufs=1, space="PSUM") as pt2,
            tc.tile_pool(name="ptp", bufs=1, space="PSUM") as ptp,
        ):
            tableT = cpool.tile([VOCAB, 32], bf16)
            nc.sync.dma_start(out=tableT, in_=tableT_d[:, :])
            sA = cpool.tile([68, 128], bf16)
            nc.sync.dma_start(out=sA, in_=sA_d[:, :])
            sB = cpool.tile([68, 32], bf16)
            nc.sync.dma_start(out=sB, in_=sB_d[:, :])
            sC = cpool.tile([62, 100], bf16)
            nc.sync.dma_start(out=sC, in_=sC_d[:, :])
            iota2d = cpool.tile([VOCAB, CHUNK + 4], f32)
            nc.sync.dma_start(out=iota2d, in_=iota_d[:, :])
            ident = cpool.tile([128, 128], f32)
            nc.sync.dma_start(out=ident, in_=ident_d[:, :])
            ones_row = cpool.tile([1, 128], bf16)
            nc.sync.dma_start(out=ones_row, in_=ones_d[:, :])

            # persistent double-buffered im2col tiles; mask rows written once
            ims_tiles = [
                imspool.tile([68, W + 2], bf16, name=f"ims{i}", tag=f"ims{i}")
                for i in range(2)
            ]
            for t in ims_tiles:
                nc.sync.dma_start(out=t[64:68, :], in_=masks_d[:, :])

            for sb in range(n_sb):
                # ids arrive as a single uint8 row; 1.5 KB DMA per superblock,
                # converted to bf16 on ACT for the PE broadcast matmul
                ids_u8 = idpool.tile([1, IDS_W], u8)
                nc.sync.dma_start(
                    out=ids_u8,
                    in_=ids_d[0:1, sb * IDS_STRIDE : sb * IDS_STRIDE + IDS_W],
                )
                ids_sb = idpool.tile([1, IDS_W], bf16)
                nc.scalar.copy(out=ids_sb, in_=ids_u8)

                p1 = stpool.tile([128, SB_CHUNKS * CHUNK_TOK], f32)
                t2 = pt2.tile([128, CHUNK_TOK, C], f32)

                for q in range(SB_CHUNKS):
                    # broadcast chars [q*W, q*W + W + 4) across partitions
                    bc = pbc.tile([VOCAB, W + 4], f32)
                    nc.tensor.matmul(
                        bc[:, :], ones_row, ids_sb[0:1, q * W : q * W + W + 4],
                        start=True, stop=True,
                    )
                    # one-hot: compare f32 PSUM broadcast against f32 iota
                    oh = ohpool.tile([VOCAB, W + 4], bf16)
                    nc.vector.tensor_tensor(
                        out=oh,
                        in0=bc[:, :],
                        in1=iota2d[:, :],
                        op=mybir.AluOpType.is_equal,
                    )
                    # gather the two im2col bands (bf16 matmuls, K=128)
                    im2p = pim.tile([64, W + 2], f32)
                    nc.tensor.matmul(
                        im2p[0:32, :], tableT, oh[:, 0 : W + 2], start=True, stop=True
                    )
                    nc.tensor.matmul(
                        im2p[32:64, :], tableT, oh[:, 1 : W + 3], start=True, stop=True
                    )
                    ims = ims_tiles[(sb * SB_CHUNKS + q) % 2]
                    nc.scalar.copy(out=ims[0:64, :], in_=im2p[:, :])

                    # conv: 3 matmuls, masks+bias folded in
                    t1 = pt1.tile([128, CHUNK_TOK, C], f32)
                    nc.tensor.matmul(
                        t1[:, :, :], sA, ims[0:68, 0:W], start=True, stop=False,
                        skip_group_check=True,
                    )
                    nc.tensor.matmul(
                        t1[0:100, :, :], sC, ims[0:62, 2 : W + 2], start=False,
                        stop=True, skip_group_check=True,
                    )
                    nc.tensor.matmul(
                        t2[32 * q : 32 * q + 32, :, :], sB, ims[0:68, 0:W],
                        start=True, stop=True, skip_group_check=True,
                        tile_position=(0, 32 * q),
                    )
                    # max-pool over the 24-wide window (poisoned tails lose)
                    nc.vector.reduce_max(
                        out=p1[:, q * CHUNK_TOK : (q + 1) * CHUNK_TOK],
                        in_=t1[:, :, :],
                        axis=mybir.AxisListType.X,
                    )

                p2 = stpool.tile([128, CHUNK_TOK], f32)
                nc.vector.reduce_max(
                    out=p2, in_=t2[:, :, :], axis=mybir.AxisListType.X
                )

                tp1 = ptp.tile([SB_TOK, 128], f32)
                nc.tensor.transpose(tp1[:, :], p1[:, :], ident[:, :])
                tp2 = ptp.tile([CHUNK_TOK, 128], f32)
                nc.tensor.transpose(tp2[:, :], p2[:, :], ident[:, :])

                ot = outpool.tile([SB_TOK, 150], u8)
                relu = mybir.ActivationFunctionType.Relu
                # T1 cols: 0:50 y3 | 50:100 y4 | 100:128 y2a
                # quantize: uint8 = Relu(32*x); host divides by 32.
                nc.scalar.activation(ot[:, 50:150], tp1[:, 0:100], relu, scale=QSCALE)
                nc.scalar.activation(ot[:, 0:28], tp1[:, 100:128], relu, scale=QSCALE)
                tp2s = outpool.tile([CHUNK_TOK, 128], u8)
                nc.scalar.activation(tp2s[:, :], tp2[:, :], relu, scale=QSCALE)
                for q in range(SB_CHUNKS):
                    # DMA (not ACT): engines can't write at partition offset 16
                    nc.sync.dma_start(
                        out=ot[q * CHUNK_TOK : (q + 1) * CHUNK_TOK, 28:50],
                        in_=tp2s[:, 32 * q : 32 * q + 22],
                    )
                nc.sync.dma_start(
                    out=out_d[sb * SB_TOK : (sb + 1) * SB_TOK, :], in_=ot
                )
    nc.finalize()
    return nc


def _consts_key(consts):
    import hashlib

    h = hashlib.sha1()
    for k in sorted(consts):
        h.update(np.ascontiguousarray(consts[k]).tobytes())
    return h.hexdigest()


def _make_runner(nc):
    """Build a cached jit(shard_map(bass_exec)) callable for 8 cores.

    Same lowering path run_bass_kernel_spmd takes under axon
    (bass2jax -> neuronx_cc_hook -> PJRT custom call), but built once and
    reused, and without shipping a zero-filled output operand: the NEFF
    writes the custom-call result buffer and this kernel stores every
    output element.
    """
    import os

    import jax
    from jax.experimental.shard_map import shard_map
    from jax.sharding import Mesh, PartitionSpec
    from concourse import bass2jax

    try:
        # persist the compiled executable (NEFF included) across processes
        jax.config.update(
            "jax_compilation_cache_dir",
            os.path.expanduser("~/.cache/jax_bass_charcnn"),
        )
        jax.config.update("jax_persistent_cache_min_compile_time_secs", 10)
    except Exception:
        pass

    bass2jax.install_neuronx_cc_hook()
    devices = jax.devices()[:N_CORES]
    assert len(devices) >= N_CORES, f"need {N_CORES} cores, have {len(devices)}"
    mesh = Mesh(np.asarray(devices), ("core",))
    out_aval = jax.core.ShapedArray((TOK_PER_CORE, 3 * F), np.uint8)

    def _body(ids):
        outs = bass2jax._bass_exec_p.bind(
            ids,
            bass2jax.partition_id_tensor(),
            out_avals=(out_aval,),
            in_names=("ids", "partition_id"),
            out_names=("out",),
            lowering_input_output_aliases=(),
            sim_require_finite=True,
            sim_require_nnan=True,
            nc=nc,
        )
        return outs[0]

    fn = jax.jit(
        shard_map(
            _body,
            mesh=mesh,
            in_specs=(PartitionSpec("core"),),
            out_specs=PartitionSpec("core"),
            check_rep=False,
        )
    )
    # AOT-compile once; the Compiled object's call skips the python pjit
    # dispatch machinery (~0.4 ms/call with numpy inputs)
    return fn.lower(
        jax.ShapeDtypeStruct((N_CORES, IDS_LEN), np.uint8)
    ).compile()


_IDS_BUF = [None]


def _pack_ids(x):
    # jax copies np inputs at dispatch time, so reusing one buffer is safe
    ids = _IDS_BUF[0]
    if ids is None:
        ids = _IDS_BUF[0] = np.zeros((N_CORES, IDS_LEN), np.uint8)
    ids[:, :CHARS_PER_CORE] = x.reshape(N_CORES, CHARS_PER_CORE).astype(np.uint8)
    return ids


# Double-buffered f32 output: reusing warm pages saves ~6 ms/call of
# page-fault cost vs a fresh 19.7 MB allocation (single-core host).
# Two buffers alternate so the previously returned array stays valid.
_OUT_BUFS = [None, None]
_OUT_IDX = [0]


def _dequant(out_u8):
    i = _OUT_IDX[0]
    _OUT_IDX[0] = 1 - i
    buf = _OUT_BUFS[i]
    if buf is None:
        buf = _OUT_BUFS[i] = np.empty((B * S, 3 * F), np.float32)
    np.multiply(
        out_u8.reshape(B * S, 3 * F),
        np.float32(1.0 / QSCALE),
        dtype=np.float32,
        out=buf,
    )
    return buf.reshape(B, S, 3 * F)


def kernel(x, emb_table, w2, b2, w3, b3, w4, b4):
    x = np.asarray(x)
    assert x.shape == (B, S, C), (x.shape, x.dtype)
    consts = _host_constants(emb_table, w2, b2, w3, b3, w4, b4)
    key = _consts_key(consts)
    entry = _CACHE.get(key)
    if entry is None:
        nc = _build(consts)
        entry = {"nc": nc, "fn": None}
        _CACHE[key] = entry

    ids = _pack_ids(x)

    if entry["fn"] is not False:
        try:
            if entry["fn"] is None:
                entry["fn"] = _make_runner(entry["nc"])
            out = np.asarray(entry["fn"](ids))
            return _dequant(out)
        except Exception:
            import sys
            import traceback

            traceback.print_exc(file=sys.stderr)
            print(
                "kernel: fast runner failed; falling back to run_bass_kernel_spmd",
                file=sys.stderr,
            )
            entry["fn"] = False  # fall back to the stock spmd path below

    from concourse.bass_utils import run_bass_kernel_spmd

    in_maps = [{"ids": ids[c : c + 1]} for c in range(N_CORES)]
    res = run_bass_kernel_spmd(entry["nc"], in_maps, core_ids=list(range(N_CORES)))
    outs = [
        np.multiply(r["out"], np.float32(1.0 / QSCALE), dtype=np.float32).reshape(B // N_CORES, S, 3 * F)
        for r in res.results
    ]
    return np.concatenate(outs, axis=0)


# revision 16
# speedup vs baseline: 1.1736x; 1.1736x over previous
"""CharCNN encoder kernel for Trainium2 (8 NeuronCores, data-parallel).

Strategy (per core, 4096 tokens = 98304 chars):
  - ids arrive as a single [1, 98308] uint8 row (96 KB/core, not
    partition-replicated on host); ACT converts to bf16 and a K=1
    ones-matmul broadcasts each 388-char chunk across the 128
    partitions into PSUM.
  - one-hot gather: OH[v,c] = (ids[c]==v) built on DVE (is_equal of the
    f32 PSUM broadcast vs an f32 iota), then E = emb_table.T @ OH on the
    PE (gather-as-matmul, K=128 vocab).
  - two shifted gather matmuls build a 2-band im2col directly in PSUM:
    rows [0:30) = E[:,c], rows [32:62) = E[:,c+1] (offset 32 required by
    PE tile_position rules; gap rows zeroed via zero-padded stationary).
  - conv = 3 bf16 matmuls on the im2col (K<=68) with mask rows (-1e9 at
    invalid window positions) and a ones row (bias) folded into the
    stationary operand.
  - max-pool = DVE windowed reduce_max (window 24, poisoned tails lose).
  - PE transpose + ACT relu-copies assemble (token, 150) rows quantized
    to uint8 (y*32, rounds to nearest on the store); the host divides
    by 32. Quantization adds ~0.5% relative error against a 2% budget
    and quarters the device->host fetch vs f32.

Host side: one jax.jit(shard_map(bass_exec)) callable is built and
cached per weight-set; warm calls reuse the compiled executable, so the
per-call cost is ids H2D (0.8 MB total) + execute + uint8 output D2H
(4.9 MB) — all dominated by the transport round trips, not device time.
"""

import numpy as np
import ml_dtypes

BF16 = ml_dtypes.bfloat16

VOCAB = 128
D = 30  # embed
F = 50  # filters per ksize
B, S, C = 64, 512, 24
N_CORES = 8
TOK_PER_CORE = (B // N_CORES) * S  # 4096
CHARS_PER_CORE = TOK_PER_CORE * C  # 98304

CHUNK_TOK = 16          # tokens per chunk
CHUNK = CHUNK_TOK * C   # 384 chars per chunk
SB_CHUNKS = 4           # chunks per superblock
SB_TOK = SB_CHUNKS * CHUNK_TOK  # 64 tokens
N_SB = TOK_PER_CORE // SB_TOK   # 64 superblocks
IDS_STRIDE = SB_CHUNKS * CHUNK  # 1536
IDS_W = IDS_STRIDE + 4          # 1540 (4-char halo for shifted reads)
IDS_LEN = CHARS_PER_CORE + 4    # 98308

NEG = -1.0e9
QSCALE = 32.0  # uint8 output quantization: u8 = Relu(32*x), max |y| ~3.2 << 8

_CACHE = {}


def _host_constants(emb_table, w2, b2, w3, b3, w4, b4):
    """Pack conv weights into PE stationary operands (see kernel docstring)."""
    emb = np.asarray(emb_table, np.float32)
    w2 = np.asarray(w2, np.float32)
    w3 = np.asarray(w3, np.float32)
    w4 = np.asarray(w4, np.float32)
    b2 = np.asarray(b2, np.float32)
    b3 = np.asarray(b3, np.float32)
    b4 = np.asarray(b4, np.float32)

    # gather stationary: (vocab, 32), cols 30:32 zero
    tableT = np.zeros((VOCAB, 32), np.float32)
    tableT[:, :D] = emb

    # im2col row layout (68 rows):
    #   0:30   band0 = E[:, c]      (j=0)
    #   30:32  zero
    #   32:62  band1 = E[:, c+1]    (j=1)
    #   62:64  zero
    #   64     mask l==21, 65 mask l==22, 66 mask l==23, 67 ones (bias)
    # T1 col layout: 0:50 y3 | 50:100 y4 | 100:128 y2a (w2 filters 0:28)
    sA = np.zeros((68, 128), np.float32)
    for j in (0, 1):
        r = 32 * j
        # w?[f, d, j] -> rows r+d, col f
        sA[r : r + D, 0:50] = w3[:, :, j].T
        sA[r : r + D, 50:100] = w4[:, :, j].T
        sA[r : r + D, 100:128] = w2[:28, :, j].T
    sA[64, 50:100] = NEG            # l=21 invalid for k=4
    sA[65, 0:100] = NEG             # l=22 invalid for k=3,4
    sA[66, 0:128] = NEG             # l=23 invalid for all
    sA[67, 0:50] = b3
    sA[67, 50:100] = b4
    sA[67, 100:128] = b2[:28]

    # y2b = w2 filters 28:50, padded to 32 cols
    sB = np.zeros((68, 32), np.float32)
    for j in (0, 1):
        r = 32 * j
        sB[r : r + D, 0:22] = w2[28:, :, j].T
    sB[66, 0:22] = NEG
    sB[67, 0:22] = b2[28:]

    # shift-2 stationary: rhs = ims[0:62, c+2] -> rows 0:30 = E[:,c+2],
    # rows 32:62 = E[:,c+3]. cols 0:50 y3 (j=2), 50:100 y4 (j=2,3).
    sC = np.zeros((62, 100), np.float32)
    sC[0:D, 0:50] = w3[:, :, 2].T
    sC[0:D, 50:100] = w4[:, :, 2].T
    sC[32 : 32 + D, 50:100] = w4[:, :, 3].T

    # mask/ones rows DMA'd once into the persistent im2col tiles
    cc = np.arange(CHUNK + 2, dtype=np.int64) % C
    masks = np.zeros((4, CHUNK + 2), np.float32)
    masks[0] = (cc == 21).astype(np.float32)
    masks[1] = (cc == 22).astype(np.float32)
    masks[2] = (cc == 23).astype(np.float32)
    masks[3] = 1.0

    iota2d = np.broadcast_to(
        np.arange(VOCAB, dtype=np.float32).reshape(VOCAB, 1), (VOCAB, CHUNK + 4)
    )
    ident = np.eye(128, dtype=np.float32)
    ones_row = np.ones((1, 128), np.float32)

    return {
        "tableT": tableT.astype(BF16),
        "sA": sA.astype(BF16),
        "sB": sB.astype(BF16),
        "sC": sC.astype(BF16),
        "masks": masks.astype(BF16),
        "iota2d": np.ascontiguousarray(iota2d, dtype=np.float32),
        "ident": ident,
        "ones_row": ones_row.astype(BF16),
    }


def _build(consts, n_sb=N_SB):
    import concourse.mybir as mybir
    from concourse import bacc
    from concourse.tile import TileContext

    f32 = mybir.dt.float32
    u8 = mybir.dt.uint8
    bf16 = mybir.dt.bfloat16
    W = CHUNK  # 384

    nc = bacc.Bacc(name="charcnn")
    ids_d = nc.dram_tensor("ids", [1, IDS_LEN], u8, kind="ExternalInput")
    out_d = nc.dram_tensor("out", [n_sb * SB_TOK, 150], u8, kind="ExternalOutput")

    tableT_d = nc.inline_tensor(consts["tableT"], "tableT")
    sA_d = nc.inline_tensor(consts["sA"], "sA")
    sB_d = nc.inline_tensor(consts["sB"], "sB")
    sC_d = nc.inline_tensor(consts["sC"], "sC")
    masks_d = nc.inline_tensor(consts["masks"], "masks")
    iota_d = nc.inline_tensor(consts["iota2d"], "iota2d")
    ident_d = nc.inline_tensor(consts["ident"], "ident")
    ones_d = nc.inline_tensor(consts["ones_row"], "ones_row")

    with TileContext(nc) as tc:
        with (
            tc.tile_pool(name="consts", bufs=1) as cpool,
            tc.tile_pool(name="idsp", bufs=2) as idpool,
            tc.tile_pool(name="ohp", bufs=3) as ohpool,
            tc.tile_pool(name="imsp", bufs=1) as imspool,
            tc.tile_pool(name="stage", bufs=2) as stpool,
            tc.tile_pool(name="outp", bufs=2) as outpool,
            tc.tile_pool(name="pbc", bufs=2, space="PSUM") as pbc,
            tc.tile_pool(name="pim", bufs=1, space="PSUM") as pim,
            tc.tile_pool(name="pt1", bufs=2, space="PSUM") as pt1,
            tc.tile_pool(name="pt2", bufs=1, space="PSUM") as pt2,
            tc.tile_pool(name="ptp", bufs=1, space="PSUM") as ptp,
        ):
            tableT = cpool.tile([VOCAB, 32], bf16)
            nc.sync.dma_start(out=tableT, in_=tableT_d[:, :])
            sA = cpool.tile([68, 128], bf16)
            nc.sync.dma_start(out=sA, in_=sA_d[:, :])
            sB = cpool.tile([68, 32], bf16)
            nc.sync.dma_start(out=sB, in_=sB_d[:, :])
            sC = cpool.tile([62, 100], bf16)
            nc.sync.dma_start(out=sC, in_=sC_d[:, :])
            iota2d = cpool.tile([VOCAB, CHUNK + 4], f32)
            nc.sync.dma_start(out=iota2d, in_=iota_d[:, :])
            ident = cpool.tile([128, 128], f32)
            nc.sync.dma_start(out=ident, in_=ident_d[:, :])
            ones_row = cpool.tile([1, 128], bf16)
            nc.sync.dma_start(out=ones_row, in_=ones_d[:, :])

            # persistent double-buffered im2col tiles; mask rows written once
            ims_tiles = [
                imspool.tile([68, W + 2], bf16, name=f"ims{i}", tag=f"ims{i}")
                for i in range(2)
            ]
            for t in ims_tiles:
                nc.sync.dma_start(out=t[64:68, :], in_=masks_d[:, :])

            for sb in range(n_sb):
                # ids arrive as a single uint8 row; 1.5 KB DMA per superblock,
                # converted to bf16 on ACT for the PE broadcast matmul
                ids_u8 = idpool.tile([1, IDS_W], u8)
                nc.sync.dma_start(
                    out=ids_u8,
                    in_=ids_d[0:1, sb * IDS_STRIDE : sb * IDS_STRIDE + IDS_W],
                )
                ids_sb = idpool.tile([1, IDS_W], bf16)
                nc.scalar.copy(out=ids_sb, in_=ids_u8)

                p1 = stpool.tile([128, SB_CHUNKS * CHUNK_TOK], f32)
                t2 = pt2.tile([128, CHUNK_TOK, C], f32)

                for q in range(SB_CHUNKS):
                    # broadcast chars [q*W, q*W + W + 4) across partitions
                    bc = pbc.tile([VOCAB, W + 4], f32)
                    nc.tensor.matmul(
                        bc[:, :], ones_row, ids_sb[0:1, q * W : q * W + W + 4],
                        start=True, stop=True,
                    )
                    # one-hot: compare f32 PSUM broadcast against f32 iota
                    oh = ohpool.tile([VOCAB, W + 4], bf16)
                    nc.vector.tensor_tensor(
                        out=oh,
                        in0=bc[:, :],
                        in1=iota2d[:, :],
                        op=mybir.AluOpType.is_equal,
                    )
                    # gather the two im2col bands (bf16 matmuls, K=128)
                    im2p = pim.tile([64, W + 2], f32)
                    nc.tensor.matmul(
                        im2p[0:32, :], tableT, oh[:, 0 : W + 2], start=True, stop=True
                    )
                    nc.tensor.matmul(
                        im2p[32:64, :], tableT, oh[:, 1 : W + 3], start=True, stop=True
                    )
                    ims = ims_tiles[(sb * SB_CHUNKS + q) % 2]
                    nc.scalar.copy(out=ims[0:64, :], in_=im2p[:, :])

                    # conv: 3 matmuls, masks+bias folded in
                    t1 = pt1.tile([128, CHUNK_TOK, C], f32)
                    nc.tensor.matmul(
                        t1[:, :, :], sA, ims[0:68, 0:W], start=True, stop=False,
                        skip_group_check=True,
                    )
                    nc.tensor.matmul(
                        t1[0:100, :, :], sC, ims[0:62, 2 : W + 2], start=False,
                        stop=True, skip_group_check=True,
                    )
                    nc.tensor.matmul(
                        t2[32 * q : 32 * q + 32, :, :], sB, ims[0:68, 0:W],
                        start=True, stop=True, skip_group_check=True,
                        tile_position=(0, 32 * q),
                    )
                    # max-pool over the 24-wide window (poisoned tails lose)
                    nc.vector.reduce_max(
                        out=p1[:, q * CHUNK_TOK : (q + 1) * CHUNK_TOK],
                        in_=t1[:, :, :],
                        axis=mybir.AxisListType.X,
                    )

                p2 = stpool.tile([128, CHUNK_TOK], f32)
                nc.vector.reduce_max(
                    out=p2, in_=t2[:, :, :], axis=mybir.AxisListType.X
                )

                tp1 = ptp.tile([SB_TOK, 128], f32)
                nc.tensor.transpose(tp1[:, :], p1[:, :], ident[:, :])
                tp2 = ptp.tile([CHUNK_TOK, 128], f32)
                nc.tensor.transpose(tp2[:, :], p2[:, :], ident[:, :])

                ot = outpool.tile([SB_TOK, 150], u8)
                relu = mybir.ActivationFunctionType.Relu
                # T1 cols: 0:50 y3 | 50:100 y4 | 100:128 y2a
                # quantize: uint8 = Relu(32*x); host divides by 32.
                nc.scalar.activation(ot[:, 50:150], tp1[:, 0:100], relu, scale=QSCALE)
                nc.scalar.activation(ot[:, 0:28], tp1[:, 100:128], relu, scale=QSCALE)
                tp2s = outpool.tile([CHUNK_TOK, 128], u8)
                nc.scalar.activation(tp2s[:, :], tp2[:, :], relu, scale=QSCALE)
                for q in range(SB_CHUNKS):
                    # DMA (not ACT): engines can't write at partition offset 16
                    nc.sync.dma_start(
                        out=ot[q * CHUNK_TOK : (q + 1) * CHUNK_TOK, 28:50],
                        in_=tp2s[:, 32 * q : 32 * q + 22],
                    )
                nc.sync.dma_start(
                    out=out_d[sb * SB_TOK : (sb + 1) * SB_TOK, :], in_=ot
                )
    nc.finalize()
    return nc


def _consts_key(consts):
    import hashlib

    h = hashlib.sha1()
    for k in sorted(consts):
        h.update(np.ascontiguousarray(consts[k]).tobytes())
    return h.hexdigest()


def _make_runner(nc):
    """Build a cached jit(shard_map(bass_exec)) callable for 8 cores.

    Same lowering path run_bass_kernel_spmd takes under axon
    (bass2jax -> neuronx_cc_hook -> PJRT custom call), but built once and
    reused, and without shipping a zero-filled output operand: the NEFF
    writes the custom-call result buffer and this kernel stores every
    output element.
    """
    import os

    import jax
    from jax.experimental.shard_map import shard_map
    from jax.sharding import Mesh, PartitionSpec
    from concourse import bass2jax

    try:
        # persist the compiled executable (NEFF included) across processes
        jax.config.update(
            "jax_compilation_cache_dir",
            os.path.expanduser("~/.cache/jax_bass_charcnn"),
        )
        jax.config.update("jax_persistent_cache_min_compile_time_secs", 10)
    except Exception:
        pass

    bass2jax.install_neuronx_cc_hook()
    devices = jax.devices()[:N_CORES]
    assert len(devices) >= N_CORES, f"need {N_CORES} cores, have {len(devices)}"
    mesh = Mesh(np.asarray(devices), ("core",))
    out_aval = jax.core.ShapedArray((TOK_PER_CORE, 3 * F), np.uint8)

    def _body(ids):
        outs = bass2jax._bass_exec_p.bind(
            ids,
            bass2jax.partition_id_tensor(),
            out_avals=(out_aval,),
            in_names=("ids", "partition_id"),
            out_names=("out",),
            lowering_input_output_aliases=(),
            sim_require_finite=True,
            sim_require_nnan=True,
            nc=nc,
        )
        return outs[0]

    fn = jax.jit(
        shard_map(
            _body,
            mesh=mesh,
            in_specs=(PartitionSpec("core"),),
            out_specs=PartitionSpec("core"),
            check_rep=False,
        )
    )
    # AOT-compile once; the Compiled object's call skips the python pjit
    # dispatch machinery (~0.4 ms/call with numpy inputs)
    return fn.lower(
        jax.ShapeDtypeStruct((N_CORES, IDS_LEN), np.uint8)
    ).compile()


_IDS_BUF = [None]


def _pack_ids(x):
    # jax copies np inputs at dispatch time, so reusing one buffer is safe
    ids = _IDS_BUF[0]
    if ids is None:
        ids = _IDS_BUF[0] = np.zeros((N_CORES, IDS_LEN), np.uint8)
    ids[:, :CHARS_PER_CORE] = x.reshape(N_CORES, CHARS_PER_CORE).astype(np.uint8)
    return ids


# Double-buffered f32 output: reusing warm pages saves ~6 ms/call of
# page-fault cost vs a fresh 19.7 MB allocation (single-core host).
# Two buffers alternate so the previously returned array stays valid.
_OUT_BUFS = [None, None]
_OUT_IDX = [0]


def _dequant(out_u8):
    i = _OUT_IDX[0]
    _OUT_IDX[0] = 1 - i
    buf = _OUT_BUFS[i]
    if buf is None:
        buf = _OUT_BUFS[i] = np.empty((B * S, 3 * F), np.float32)
    np.multiply(
        out_u8.reshape(B * S, 3 * F),
        np.float32(1.0 / QSCALE),
        dtype=np.float32,
        out=buf,
    )
    return buf.reshape(B, S, 3 * F)


def kernel(x, emb_table, w2, b2, w3, b3, w4, b4):
    x = np.asarray(x)
    assert x.shape == (B, S, C), (x.shape, x.dtype)
    # exact-match weight cache: memcmp-speed comparison beats hashing and
    # skips _host_constants entirely on warm calls (~1 ms)
    raw = tuple(np.asarray(a) for a in (emb_table, w2, b2, w3, b3, w4, b4))
    entry = None
    for e in _CACHE.values():
        if all(np.array_equal(a, b) for a, b in zip(e["raw"], raw)):
            entry = e
            break
    if entry is None:
        consts = _host_constants(*raw)
        nc = _build(consts)
        entry = {"nc": nc, "fn": None, "raw": raw}
        _CACHE[len(_CACHE)] = entry

    ids = _pack_ids(x)

    if entry["fn"] is not False:
        try:
            if entry["fn"] is None:
                entry["fn"] = _make_runner(entry["nc"])
            out = np.asarray(entry["fn"](ids))
            return _dequant(out)
        except Exception:
            import sys
            import traceback

            traceback.print_exc(file=sys.stderr)
            print(
                "kernel: fast runner failed; falling back to run_bass_kernel_spmd",
                file=sys.stderr,
            )
            entry["fn"] = False  # fall back to the stock spmd path below

    from concourse.bass_utils import run_bass_kernel_spmd

    in_maps = [{"ids": ids[c : c + 1]} for c in range(N_CORES)]
    res = run_bass_kernel_spmd(entry["nc"], in_maps, core_ids=list(range(N_CORES)))
    outs = [
        np.multiply(r["out"], np.float32(1.0 / QSCALE), dtype=np.float32).reshape(B // N_CORES, S, 3 * F)
        for r in res.results
    ]
    return np.concatenate(outs, axis=0)
